# revision 14
# baseline (speedup 1.0000x reference)
"""Differential attention (nn_DifferentialAttention) on 8 TRN2 NeuronCores.

Sharding: tensor-parallel over heads. 16 diff-heads / 8 cores = 2 heads per
core. Each core:
  - computes Q^T/K^T/V for its 2 heads' column slices of Wq/Wk/Wv (form-B
    matmuls vs a host-pretransposed x^T),
  - runs both dual-softmax attentions fully on-device (exp on ACT with fused
    row-sum accumulation, combine A = r1*E1 - lam*r2*E2 via one tensor_scalar
    + one scalar_tensor_tensor, PE transposes for the A@V contraction),
  - applies the fused RMSNorm and its row-slice of Wo.
Host: sums the 8 partial out projections, stacks the A shards.

All matmuls run as float32r (full-rate fp32 on the PE for free dim >= 256).
"""

import math
import os
import sys

import numpy as np

sys.path.insert(0, "/opt/trn_rl_repo")

from contextlib import ExitStack

from concourse import bacc, bass, bass_isa, tile
from concourse import mybir
from concourse.bass_utils import run_bass_kernel_spmd
from concourse.masks import make_identity

F32 = mybir.dt.float32
F32R = mybir.dt.float32r
EXP = mybir.ActivationFunctionType.Exp
LOG = mybir.ActivationFunctionType.Ln
MULT = mybir.AluOpType.mult
ADD = mybir.AluOpType.add

D = 2048          # embed dim
T = 2048          # tokens
HD = 64           # half-head dim
NCORES = 8
HPC = 2           # heads per core
LAMBDA_INIT = 0.8 - 0.6 * math.exp(-0.3 * 0.0)   # depth 0 -> 0.2
EPS = 1e-5
EXP_SHIFT = -24.0  # constant logit shift; softmax-invariant, keeps exp small


def _r(ap):
    return ap.bitcast(F32R)


def _build_kernel():
    nc = bacc.Bacc("TRN2", target_bir_lowering=False, debug=False)

    xt = nc.dram_tensor("xt", [D, T], F32, kind="ExternalInput").ap()
    wq = nc.dram_tensor("wq", [D, HPC * 2 * HD], F32, kind="ExternalInput").ap()
    wk = nc.dram_tensor("wk", [D, HPC * 2 * HD], F32, kind="ExternalInput").ap()
    wv = nc.dram_tensor("wv", [D, HPC * 2 * HD], F32, kind="ExternalInput").ap()
    wo = nc.dram_tensor("wo", [HPC * 2 * HD, D], F32, kind="ExternalInput").ap()
    subw = nc.dram_tensor("subw", [128, 1], F32, kind="ExternalInput").ap()
    lamn = nc.dram_tensor("lamn", [128, 1], F32, kind="ExternalInput").ap()
    a_out = nc.dram_tensor("a_out", [HPC, T, T], F32, kind="ExternalOutput").ap()
    out_p = nc.dram_tensor("out_p", [T, D], F32, kind="ExternalOutput").ap()

    with tile.TileContext(nc) as tc:
        with ExitStack() as ctx:
            _emit(ctx, tc, xt, wq, wk, wv, wo, subw, lamn, a_out, out_p)

    nc.compile()
    return nc


def _emit(ctx, tc, xt, wq, wk, wv, wo, subw, lamn, a_out, out_p):
    nc = tc.nc

    # ---------------- persistent tiles (span both phases) ----------------
    pers = ctx.enter_context(tc.tile_pool(name="pers", bufs=1))
    ident = pers.tile([128, 128], F32, tag="ident")
    make_identity(nc, ident[:])
    subw_sb = pers.tile([128, 1], F32, tag="subw")
    nc.sync.dma_start(subw_sb[:], subw[:])
    shift_sb = pers.tile([128, 1], F32, tag="shift")
    nc.gpsimd.memset(shift_sb[:], EXP_SHIFT)
    eps_sb = pers.tile([128, 1], F32, tag="eps")
    nc.gpsimd.memset(eps_sb[:], EPS)
    lamn_sb = pers.tile([128, 1], F32, tag="lamn")
    nc.sync.dma_start(lamn_sb[:], lamn[:])

    # Q^T / K^T chunks: [128 qdims, T]; chunk h covers head h
    # (partitions 0:64 = half-1 dims, 64:128 = half-2 dims).
    qt = [pers.tile([128, T], F32R, tag=f"qt{m}", name=f"qt{m}") for m in range(2)]
    kt = [pers.tile([128, T], F32R, tag=f"kt{m}", name=f"kt{m}") for m in range(2)]
    # V natural layout: [128 s-in-chunk, 16 s-chunks x 256 vdims]
    v_sb = pers.tile([128, 16 * 256], F32R, tag="v_sb")

    # ---------------- phase 1: projections (form B, fp32r) ----------------
    with ExitStack() as p1:
        xtp = p1.enter_context(tc.tile_pool(name="xtp", bufs=1))
        wp = p1.enter_context(tc.tile_pool(name="wp", bufs=1))
        vtp = p1.enter_context(tc.tile_pool(name="vtp", bufs=1))
        pps = p1.enter_context(tc.tile_pool(name="pps", bufs=4, space="PSUM"))
        tps1 = p1.enter_context(tc.tile_pool(name="tps1", bufs=2, space="PSUM"))

        # all weight chunks resident: 3 proj x 16 chunks x [128, 256]
        wch = []
        for pi, wdram in enumerate((wq, wk, wv)):
            row = []
            for d in range(16):
                w_t = wp.tile([128, 256], F32R, tag=f"w{pi}_{d}", name=f"w{pi}_{d}")
                nc.sync.dma_start(w_t[:], wdram[d * 128:(d + 1) * 128, :].bitcast(F32R))
                row.append(w_t)
            wch.append(row)

        # V^T transient chunks
        vt = [vtp.tile([128, T], F32, tag=f"vt{m}", name=f"vt{m}") for m in range(2)]

        # x^T streamed in two d-halves of 8 chunks (64KB/partition resident)
        xt_t = [None] * 16
        for half in range(2):
            for d in range(half * 8, half * 8 + 8):
                x_t = xtp.tile([128, T], F32R, tag=f"xt{d % 8}", name=f"xt{d}")
                nc.sync.dma_start(x_t[:], xt[d * 128:(d + 1) * 128, :].bitcast(F32R))
                xt_t[d] = x_t
            for pi in range(3):
                for m in range(2):
                    dst = (qt, kt, vt)[pi][m]
                    for tt in range(4):
                        ps = pps.tile([128, 512], F32, tag="pps")
                        for j, d in enumerate(range(half * 8, half * 8 + 8)):
                            nc.tensor.matmul(
                                ps[:],
                                lhsT=_r(wch[pi][d][:, m * 128:(m + 1) * 128]),
                                rhs=_r(xt_t[d][:, tt * 512:(tt + 1) * 512]),
                                start=(j == 0),
                                stop=(j == 7),
                            )
                        dslice = dst[:, tt * 512:(tt + 1) * 512]
                        if half == 0:
                            nc.vector.tensor_copy(dslice, ps[:])
                        else:
                            nc.vector.tensor_add(
                                dslice, dslice.bitcast(F32), ps[:]
                            )

        # V^T -> V natural via PE transposes (32 x [128,128])
        for m in range(2):
            for s4 in range(4):
                tp = tps1.tile([128, 512], F32, tag="tps1")
                for j in range(4):
                    sb = s4 * 4 + j
                    nc.tensor.transpose(
                        tp[:, j * 128:(j + 1) * 128],
                        vt[m][:, sb * 128:(sb + 1) * 128],
                        ident[:],
                    )
                # dst: v_sb[:, (s4*4+j)*256 + m*128 : +128] for j=0..3
                dst = v_sb[:].rearrange("p (c m d) -> p c m d", c=16, m=2, d=128)[
                    :, s4 * 4:(s4 + 1) * 4, m, :
                ]
                nc.vector.tensor_copy(
                    dst, tp[:].rearrange("p (a d) -> p a d", a=4, d=128)
                )

    # ---------------- phase 2: attention ----------------
    with ExitStack() as p2:
        ep = p2.enter_context(tc.tile_pool(name="ep", bufs=2))
        ap_ = p2.enter_context(tc.tile_pool(name="ap_", bufs=2))
        t2p = p2.enter_context(tc.tile_pool(name="t2p", bufs=2))
        atp = p2.enter_context(tc.tile_pool(name="atp", bufs=1))
        zp_ = p2.enter_context(tc.tile_pool(name="zp_", bufs=4))
        wop = p2.enter_context(tc.tile_pool(name="wop", bufs=1))
        anp = p2.enter_context(tc.tile_pool(name="anp", bufs=1))
        osb = p2.enter_context(tc.tile_pool(name="osb", bufs=4))
        lps = p2.enter_context(tc.tile_pool(name="lps", bufs=2, space="PSUM"))
        tps = p2.enter_context(tc.tile_pool(name="tps", bufs=2, space="PSUM"))
        ups = p2.enter_context(tc.tile_pool(name="ups", bufs=2, space="PSUM"))

        wo_sb = [wop.tile([128, D], F32R, tag=f"wo{m}", name=f"wo{m}") for m in range(2)]
        for m in range(2):
            nc.sync.dma_start(
                wo_sb[m][:], wo[m * 128:(m + 1) * 128, :].bitcast(F32R)
            )

        attnT = [
            anp.tile([128, T], F32R, tag=f"attnT{h}", name=f"attnT{h}")
            for h in range(HPC)
        ]

        copy_flip = [0]

        def psum_copy(dst, src):
            # alternate PSUM->SBUF copies between DVE and ACT to balance load
            copy_flip[0] ^= 1
            if copy_flip[0]:
                nc.vector.tensor_copy(dst, src)
            else:
                nc.scalar.activation(
                    dst, src, mybir.ActivationFunctionType.Identity, bias=0.0
                )

        for h in range(HPC):
            for g in range(4):
                at_sb = atp.tile([128, 16 * 512], F32R, tag="at_sb")
                for u in range(4):
                    t0 = g * 512 + u * 128
                    e1 = ep.tile([128, T], F32, tag="e1")
                    e2 = ep.tile([128, T], F32, tag="e2")
                    zp = zp_.tile([128, 4], F32, tag="zp")
                    for sh in range(2):
                        l1 = lps.tile([128, 1024], F32, tag="lps")
                        l2 = lps.tile([128, 1024], F32, tag="lps")
                        for st in range(2):
                            s0 = sh * 1024 + st * 512
                            nc.tensor.matmul(
                                l1[:, st * 512:(st + 1) * 512],
                                lhsT=_r(qt[h][0:64, t0:t0 + 128]),
                                rhs=_r(kt[h][0:64, s0:s0 + 512]),
                                start=True,
                                stop=True,
                            )
                            nc.tensor.matmul(
                                l2[:, st * 512:(st + 1) * 512],
                                lhsT=_r(qt[h][64:128, t0:t0 + 128]),
                                rhs=_r(kt[h][64:128, s0:s0 + 512]),
                                start=True,
                                stop=True,
                            )
                        nc.scalar.activation(
                            e1[:, sh * 1024:(sh + 1) * 1024], l1[:], EXP,
                            bias=shift_sb[:], accum_out=zp[:, sh:sh + 1],
                        )
                        nc.scalar.activation(
                            e2[:, sh * 1024:(sh + 1) * 1024], l2[:], EXP,
                            bias=shift_sb[:], accum_out=zp[:, 2 + sh:3 + sh],
                        )
                    # z = [Z1, Z2]; r = 1/z ; sc = -lam * r2
                    zs = zp_.tile([128, 2], F32, tag="zs")
                    zpv = zp[:].rearrange("p (h s) -> p h s", h=2, s=2)
                    nc.vector.tensor_add(zs[:], zpv[:, :, 0], zpv[:, :, 1])
                    rr = zp_.tile([128, 2], F32, tag="rr")
                    nc.vector.reciprocal(rr[:], zs[:])
                    sc = zp_.tile([128, 1], F32, tag="sc")
                    nc.vector.tensor_mul(sc[:], rr[:, 1:2], lamn_sb[:])
                    # T2 = E2 * sc ; A = (E1 * r1) + T2
                    t2 = t2p.tile([128, T], F32, tag="t2")
                    nc.gpsimd.tensor_scalar(t2[:], e2[:], sc[:], None, op0=MULT)
                    a_t = ap_.tile([128, T], F32, tag="a_t")
                    nc.vector.scalar_tensor_tensor(
                        a_t[:], in0=e1[:], scalar=rr[:, 0:1], in1=t2[:],
                        op0=MULT, op1=ADD,
                    )
                    nc.sync.dma_start(a_out[h, t0:t0 + 128, :], a_t[:])
                    # transpose A -> at_sb[s, t]
                    for k4 in range(4):
                        tp = tps.tile([128, 512], F32, tag="tps")
                        for j in range(4):
                            sb = k4 * 4 + j
                            nc.tensor.transpose(
                                tp[:, j * 128:(j + 1) * 128],
                                a_t[:, sb * 128:(sb + 1) * 128],
                                ident[:],
                            )
                        dst = at_sb[:].rearrange(
                            "p (c u d) -> p c u d", c=16, u=4, d=128
                        )[:, k4 * 4:(k4 + 1) * 4, u, :]
                        psum_copy(
                            dst, tp[:].rearrange("p (a d) -> p a d", a=4, d=128)
                        )
                # A @ V for this t-group
                u_ps = ups.tile([128, 512], F32, tag="ups")
                for ch in range(16):
                    nc.tensor.matmul(
                        u_ps[:],
                        lhsT=_r(v_sb[:, ch * 256 + h * 128:ch * 256 + (h + 1) * 128]),
                        rhs=_r(at_sb[:, ch * 512:(ch + 1) * 512]),
                        start=(ch == 0),
                        stop=(ch == 15),
                    )
                nc.vector.tensor_copy(attnT[h][:, g * 512:(g + 1) * 512], u_ps[:])

            # fused RMSNorm over the 128 head dims (partition axis)
            sq = t2p.tile([128, T], F32, tag="t2")
            atv = attnT[h][:].bitcast(F32)
            nc.vector.tensor_mul(sq[:], atv, atv)
            ssum = ep.tile([128, T], F32, tag="e1")
            nc.gpsimd.partition_all_reduce(
                ssum[:], sq[:], channels=128, reduce_op=bass_isa.ReduceOp.add
            )
            lnv = ep.tile([128, T], F32, tag="e2")
            nc.scalar.activation(
                lnv[:], ssum[:], LOG, bias=eps_sb[:], scale=1.0 / 128.0
            )
            rinv = t2p.tile([128, T], F32, tag="t2")
            nc.scalar.activation(rinv[:], lnv[:], EXP, scale=-0.5)
            # normed^T = (attnT * subw) * rinv   (subw already includes 1-lambda_init)
            nc.vector.scalar_tensor_tensor(
                attnT[h][:], in0=atv, scalar=subw_sb[:], in1=rinv[:],
                op0=MULT, op1=MULT,
            )

        # out_p = normed @ Wo_slice
        for tt in range(16):
            for n in range(4):
                o_ps = ups.tile([128, 512], F32, tag="ups")
                for m in range(2):
                    nc.tensor.matmul(
                        o_ps[:],
                        lhsT=_r(attnT[m][:, tt * 128:(tt + 1) * 128]),
                        rhs=_r(wo_sb[m][:, n * 512:(n + 1) * 512]),
                        start=(m == 0),
                        stop=(m == 1),
                    )
                o_sb = osb.tile([128, 512], F32, tag="o_sb")
                psum_copy(o_sb[:], o_ps[:])
                nc.sync.dma_start(
                    out_p[tt * 128:(tt + 1) * 128, n * 512:(n + 1) * 512], o_sb[:]
                )


_NC_CACHE = None


def _get_nc():
    global _NC_CACHE
    if _NC_CACHE is None:
        _NC_CACHE = _build_kernel()
    return _NC_CACHE


def kernel(x, Wq, Wk, Wv, Wo, lambda_q1, lambda_k1, lambda_q2, lambda_k2,
           subln_weight):
    x = np.asarray(x, dtype=np.float32)
    Wq = np.asarray(Wq, dtype=np.float32)
    Wk = np.asarray(Wk, dtype=np.float32)
    Wv = np.asarray(Wv, dtype=np.float32)
    Wo = np.asarray(Wo, dtype=np.float32)

    lam = float(
        np.exp(np.sum(np.asarray(lambda_q1, np.float64) * np.asarray(lambda_k1, np.float64)))
        - np.exp(np.sum(np.asarray(lambda_q2, np.float64) * np.asarray(lambda_k2, np.float64)))
        + LAMBDA_INIT
    )
    subw = (np.asarray(subln_weight, np.float32) * np.float32(1.0 - LAMBDA_INIT))
    subw = np.ascontiguousarray(subw.reshape(128, 1))
    lamn = np.full((128, 1), -lam, dtype=np.float32)

    xtv = np.ascontiguousarray(x[0].T)  # [D, T]

    in_maps = []
    for c in range(NCORES):
        lo, hi = c * 256, (c + 1) * 256
        in_maps.append({
            "xt": xtv,
            "wq": np.ascontiguousarray(Wq[:, lo:hi]),
            "wk": np.ascontiguousarray(Wk[:, lo:hi]),
            "wv": np.ascontiguousarray(Wv[:, lo:hi]),
            "wo": np.ascontiguousarray(Wo[lo:hi, :]),
            "subw": subw,
            "lamn": lamn,
        })

    nc = _get_nc()
    res = run_bass_kernel_spmd(nc, in_maps, list(range(NCORES))).results

    out = np.zeros((T, D), dtype=np.float32)
    A = np.empty((1, 2 * NCORES, T, T), dtype=np.float32)
    for c in range(NCORES):
        out += res[c]["out_p"]
        A[0, 2 * c:2 * c + 2] = res[c]["a_out"]
    return out.reshape(1, T, D), A


# revision 29
# speedup vs baseline: 32030.8568x; 32030.8568x over previous
"""Differential attention (nn_DifferentialAttention) on 8 TRN2 NeuronCores.

Sharding: tensor-parallel over heads. 16 diff-heads / 8 cores = 2 heads per
core. Each core:
  - computes Q^T/K^T/V for its 2 heads' column slices of Wq/Wk/Wv (form-B
    matmuls vs a host-pretransposed x^T),
  - runs both dual-softmax attentions fully on-device (exp on ACT with fused
    row-sum accumulation, combine A = r1*E1 - lam*r2*E2 via one tensor_scalar
    + one scalar_tensor_tensor, PE transposes for the A@V contraction),
  - applies the fused RMSNorm and its row-slice of Wo.
Host: sums the 8 partial out projections, stacks the A shards.

All matmuls run as float32r (full-rate fp32 on the PE for free dim >= 256).
"""

import math
import os
import sys

import numpy as np

sys.path.insert(0, "/opt/trn_rl_repo")

from contextlib import ExitStack

from concourse import bacc, bass, bass_isa, tile
from concourse import mybir
from concourse.bass_utils import run_bass_kernel_spmd
from concourse.masks import make_identity

F32 = mybir.dt.float32
F32R = mybir.dt.float32r
EXP = mybir.ActivationFunctionType.Exp
LOG = mybir.ActivationFunctionType.Ln
MULT = mybir.AluOpType.mult
ADD = mybir.AluOpType.add

D = 2048          # embed dim
T = 2048          # tokens
HD = 64           # half-head dim
NCORES = 8
HPC = 2           # heads per core
LAMBDA_INIT = 0.8 - 0.6 * math.exp(-0.3 * 0.0)   # depth 0 -> 0.2
EPS = 1e-5
EXP_SHIFT = -24.0  # constant logit shift; softmax-invariant, keeps exp small


def _r(ap):
    return ap.bitcast(F32R)


def _build_kernel():
    nc = bacc.Bacc("TRN2", target_bir_lowering=False, debug=False)

    xt = nc.dram_tensor("xt", [D, T], F32, kind="ExternalInput").ap()
    wq = nc.dram_tensor("wq", [D, HPC * 2 * HD], F32, kind="ExternalInput").ap()
    wk = nc.dram_tensor("wk", [D, HPC * 2 * HD], F32, kind="ExternalInput").ap()
    wv = nc.dram_tensor("wv", [D, HPC * 2 * HD], F32, kind="ExternalInput").ap()
    wo = nc.dram_tensor("wo", [HPC * 2 * HD, D], F32, kind="ExternalInput").ap()
    subw = nc.dram_tensor("subw", [128, 1], F32, kind="ExternalInput").ap()
    lamn = nc.dram_tensor("lamn", [128, 1], F32, kind="ExternalInput").ap()
    iden = nc.dram_tensor("iden", [128, 128], F32, kind="ExternalInput").ap()
    a_out = nc.dram_tensor("a_out", [HPC, T, T], F32, kind="ExternalOutput").ap()
    out_p = nc.dram_tensor("out_p", [T, D], F32, kind="ExternalOutput").ap()

    with tile.TileContext(nc) as tc:
        with ExitStack() as ctx:
            _emit(ctx, tc, xt, wq, wk, wv, wo, subw, lamn, iden, a_out, out_p)

    nc.compile()
    return nc


def _emit(ctx, tc, xt, wq, wk, wv, wo, subw, lamn, iden, a_out, out_p):
    nc = tc.nc

    # ---------------- persistent tiles (span both phases) ----------------
    pers = ctx.enter_context(tc.tile_pool(name="pers", bufs=1))
    ident = pers.tile([128, 128], F32, tag="ident")
    nc.sync.dma_start(ident[:], iden[:])
    subw_sb = pers.tile([128, 1], F32, tag="subw")
    nc.sync.dma_start(subw_sb[:], subw[:])
    shift_sb = pers.tile([128, 1], F32, tag="shift")
    nc.gpsimd.memset(shift_sb[:], EXP_SHIFT)
    eps_sb = pers.tile([128, 1], F32, tag="eps")
    nc.gpsimd.memset(eps_sb[:], EPS)
    lamn_sb = pers.tile([128, 1], F32, tag="lamn")
    nc.sync.dma_start(lamn_sb[:], lamn[:])

    # Q^T / K^T chunks: [128 qdims, T]; chunk h covers head h
    # (partitions 0:64 = half-1 dims, 64:128 = half-2 dims).
    qt = [pers.tile([128, T], F32R, tag=f"qt{m}", name=f"qt{m}") for m in range(2)]
    kt = [pers.tile([128, T], F32R, tag=f"kt{m}", name=f"kt{m}") for m in range(2)]
    # V natural layout: [128 s-in-chunk, 16 s-chunks x 256 vdims]
    v_sb = pers.tile([128, 16 * 256], F32R, tag="v_sb")

    # ---------------- phase 1: projections (form B, fp32r) ----------------
    with ExitStack() as p1:
        xtp = p1.enter_context(tc.tile_pool(name="xtp", bufs=1))
        wp = p1.enter_context(tc.tile_pool(name="wp", bufs=1))
        vtp = p1.enter_context(tc.tile_pool(name="vtp", bufs=1))
        pps = p1.enter_context(tc.tile_pool(name="pps", bufs=4, space="PSUM"))
        tps1 = p1.enter_context(tc.tile_pool(name="tps1", bufs=2, space="PSUM"))

        # x^T first half on the sync HWDGE ring first (first matmul needs
        # xt chunk 0), weight chunks on the scalar ring in parallel
        xt_t = [None] * 16
        for d in range(8):
            x_t = xtp.tile([128, T], F32R, tag=f"xt{d % 8}", name=f"xt{d}")
            eng = nc.sync if d % 2 == 0 else nc.scalar
            eng.dma_start(x_t[:], xt[d * 128:(d + 1) * 128, :].bitcast(F32R))
            xt_t[d] = x_t

        # all weight chunks resident: 3 proj x 16 chunks x [128, 256]
        wch = []
        for pi, wdram in enumerate((wq, wk, wv)):
            row = []
            for d in range(16):
                w_t = wp.tile([128, 256], F32R, tag=f"w{pi}_{d}", name=f"w{pi}_{d}")
                nc.gpsimd.dma_start(
                    w_t[:], wdram[d * 128:(d + 1) * 128, :].bitcast(F32R)
                )
                row.append(w_t)
            wch.append(row)

        # V^T transient chunks
        vt = [vtp.tile([128, T], F32, tag=f"vt{m}", name=f"vt{m}") for m in range(2)]

        for half in range(2):
            for d in range(half * 8, half * 8 + 8):
                if xt_t[d] is None:
                    x_t = xtp.tile([128, T], F32R, tag=f"xt{d % 8}", name=f"xt{d}")
                    nc.sync.dma_start(
                        x_t[:], xt[d * 128:(d + 1) * 128, :].bitcast(F32R)
                    )
                    xt_t[d] = x_t
            for pi in range(3):
                for m in range(2):
                    dst = (qt, kt, vt)[pi][m]
                    for tt in range(4):
                        ps = pps.tile([128, 512], F32, tag="pps")
                        for j, d in enumerate(range(half * 8, half * 8 + 8)):
                            nc.tensor.matmul(
                                ps[:],
                                lhsT=_r(wch[pi][d][:, m * 128:(m + 1) * 128]),
                                rhs=_r(xt_t[d][:, tt * 512:(tt + 1) * 512]),
                                start=(j == 0),
                                stop=(j == 7),
                            )
                        dslice = dst[:, tt * 512:(tt + 1) * 512]
                        if half == 0:
                            nc.vector.tensor_copy(dslice, ps[:])
                        else:
                            nc.vector.tensor_add(
                                dslice, dslice.bitcast(F32), ps[:]
                            )

        # V^T -> V natural via PE transposes
        for m in range(2):
            for s4 in range(4):
                tp = tps1.tile([128, 512], F32, tag="tps1")
                for j in range(4):
                    sb = s4 * 4 + j
                    nc.tensor.transpose(
                        tp[:, j * 128:(j + 1) * 128],
                        vt[m][:, sb * 128:(sb + 1) * 128],
                        ident[:],
                    )
                # dst: v_sb[:, (s4*4+j)*256 + m*128 : +128] for j=0..3
                dst = v_sb[:].rearrange("p (c m d) -> p c m d", c=16, m=2, d=128)[
                    :, s4 * 4:(s4 + 1) * 4, m, :
                ]
                nc.vector.tensor_copy(
                    dst, tp[:].rearrange("p (a d) -> p a d", a=4, d=128)
                )

    # ---------------- phase 2: attention ----------------
    with ExitStack() as p2:
        anp = p2.enter_context(tc.tile_pool(name="anp", bufs=1))
        ups = p2.enter_context(tc.tile_pool(name="ups", bufs=2, space="PSUM"))

        attnT = [
            anp.tile([128, T], F32R, tag=f"attnT{h}", name=f"attnT{h}")
            for h in range(HPC)
        ]

        copy_flip = [0]

        def psum_copy(dst, src):
            # alternate PSUM->SBUF copies between DVE and ACT to balance load
            copy_flip[0] ^= 1
            if copy_flip[0]:
                nc.vector.tensor_copy(dst, src)
            else:
                nc.scalar.activation(
                    dst, src, mybir.ActivationFunctionType.Identity, bias=0.0
                )

        with ExitStack() as p2a:
            ep = p2a.enter_context(tc.tile_pool(name="ep", bufs=2))
            ap_ = p2a.enter_context(tc.tile_pool(name="ap_", bufs=2))
            atp = p2a.enter_context(tc.tile_pool(name="atp", bufs=1))
            zp_ = p2a.enter_context(tc.tile_pool(name="zp_", bufs=4))
            lps = p2a.enter_context(tc.tile_pool(name="lps", bufs=2, space="PSUM"))
            tps = p2a.enter_context(tc.tile_pool(name="tps", bufs=2, space="PSUM"))
            wop = p2a.enter_context(tc.tile_pool(name="wop", bufs=1))
            osb = p2a.enter_context(tc.tile_pool(name="osb", bufs=4))

            wo_sb = [
                wop.tile([128, D], F32R, tag=f"wo{m}", name=f"wo{m}")
                for m in range(2)
            ]
            for m in range(2):
                nc.sync.dma_start(
                    wo_sb[m][:], wo[m * 128:(m + 1) * 128, :].bitcast(F32R)
                )

            def combine_stage(pend):
                """B = E1 - (lam*Z1/Z2)*E2 (one STT); A = B * r1; A -> DRAM.
                Returns the A tile for the interleaved transpose groups."""
                if pend is None:
                    return None
                h, u, t0, e1, e2, rr, sc2 = pend
                b_t = ap_.tile([128, T], F32, tag="b_t", bufs=1)
                nc.vector.scalar_tensor_tensor(
                    b_t[:], in0=e2[:], scalar=sc2[:], in1=e1[:],
                    op0=MULT, op1=ADD,
                )
                a_t = ap_.tile([128, T], F32, tag="a_t")
                nc.vector.tensor_scalar(
                    a_t[:], b_t[:], rr[:, 0:1], None, op0=MULT
                )
                nc.sync.dma_start(a_out[h, t0:t0 + 128, :], a_t[:])
                return (u, a_t)

            def transpose_group(done, at_sb, k4):
                """4 PE transposes of the previous t-sub's A + one strided
                PSUM->SBUF copy. Emitted between logits pairs so every HAM
                activity window still contains real matmuls."""
                if done is None:
                    return
                u, a_t = done
                tp = tps.tile([128, 512], F32, tag="tps")
                for j in range(4):
                    sb = k4 * 4 + j
                    nc.tensor.transpose(
                        tp[:, j * 128:(j + 1) * 128],
                        a_t[:, sb * 128:(sb + 1) * 128],
                        ident[:],
                    )
                dst = at_sb[:].rearrange(
                    "p (c u d) -> p c u d", c=16, u=4, d=128
                )[:, k4 * 4:(k4 + 1) * 4, u, :]
                psum_copy(
                    dst, tp[:].rearrange("p (a d) -> p a d", a=4, d=128)
                )

            # RMSNorm split in two stages so the gpsimd partition-reduce of
            # head 0 overlaps head 1's attention, and the ACT Ln/Exp ops for
            # both heads batch together (2 table switches instead of 4).
            ssum = [None] * HPC

            for h in range(HPC):
                ssum[h] = anp.tile([128, T], F32, tag=f"ssum{h}", name=f"ssum{h}")

            def rms_reduce_slice(h, g):
                # per-t-group RMS partition-reduce, emitted right after the
                # AV copy of that group so the gpsimd work is spread across
                # the attention phase instead of serializing at the end
                gs = slice(g * 512, (g + 1) * 512)
                sq = zp_.tile([128, 512], F32, tag="sq", bufs=2)
                atv = attnT[h][:, gs].bitcast(F32)
                nc.vector.tensor_mul(sq[:], atv, atv)
                nc.gpsimd.partition_all_reduce(
                    ssum[h][:, gs], sq[:], channels=128,
                    reduce_op=bass_isa.ReduceOp.add,
                )

            def rms_apply():
                # halved ops so the tail chain pipelines behind the second
                # partition-reduce half
                lnvs = []
                for h in range(HPC):
                    lnv = ep.tile([128, T], F32, tag="e2")
                    for k in range(2):
                        nc.scalar.activation(
                            lnv[:, k * 1024:(k + 1) * 1024],
                            ssum[h][:, k * 1024:(k + 1) * 1024], LOG,
                            bias=eps_sb[:], scale=1.0 / 128.0,
                        )
                    lnvs.append(lnv)
                rinvs = []
                for h in range(HPC):
                    rinv = ep.tile([128, T], F32, tag="e1")
                    for k in range(2):
                        nc.scalar.activation(
                            rinv[:, k * 1024:(k + 1) * 1024],
                            lnvs[h][:, k * 1024:(k + 1) * 1024], EXP,
                            scale=-0.5,
                        )
                    rinvs.append(rinv)
                for h in range(HPC):
                    nc.vector.scalar_tensor_tensor(
                        attnT[h][:], in0=attnT[h][:].bitcast(F32),
                        scalar=subw_sb[:], in1=rinvs[h][:],
                        op0=MULT, op1=MULT,
                    )

            # The transpose groups of t-sub u are interleaved between the
            # logits pairs of t-sub u+1 (crossing g/h boundaries), so the PE
            # stream never has a matmul-free HAM window.
            done = None           # (u, a_t) whose transposes are pending
            done_at = None        # the at_sb those transposes write into
            for h in range(HPC):
                for g in range(4):
                    at_sb = atp.tile([128, 16 * 512], F32R, tag="at_sb")
                    pend = None
                    for u in range(4):
                        t0 = g * 512 + u * 128
                        e1 = ep.tile([128, T], F32, tag="e1")
                        e2 = ep.tile([128, T], F32, tag="e2")
                        zp = zp_.tile([128, 4], F32, tag="zp")
                        for sh in range(2):
                            l1 = lps.tile([128, 1024], F32, tag="lps")
                            l2 = lps.tile([128, 1024], F32, tag="lps")
                            for st in range(2):
                                s0 = sh * 1024 + st * 512
                                nc.tensor.matmul(
                                    l1[:, st * 512:(st + 1) * 512],
                                    lhsT=_r(qt[h][0:64, t0:t0 + 128]),
                                    rhs=_r(kt[h][0:64, s0:s0 + 512]),
                                    start=True,
                                    stop=True,
                                )
                                nc.tensor.matmul(
                                    l2[:, st * 512:(st + 1) * 512],
                                    lhsT=_r(qt[h][64:128, t0:t0 + 128]),
                                    rhs=_r(kt[h][64:128, s0:s0 + 512]),
                                    start=True,
                                    stop=True,
                                )
                                transpose_group(done, done_at, sh * 2 + st)
                            nc.scalar.activation(
                                e1[:, sh * 1024:(sh + 1) * 1024], l1[:], EXP,
                                bias=shift_sb[:], accum_out=zp[:, sh:sh + 1],
                            )
                            nc.scalar.activation(
                                e2[:, sh * 1024:(sh + 1) * 1024], l2[:], EXP,
                                bias=shift_sb[:], accum_out=zp[:, 2 + sh:3 + sh],
                            )
                        # z = [Z1, Z2]; r = 1/z ; sc2 = -lam*Z1/Z2
                        zs = zp_.tile([128, 2], F32, tag="zs")
                        zpv = zp[:].rearrange("p (h s) -> p h s", h=2, s=2)
                        nc.vector.tensor_add(zs[:], zpv[:, :, 0], zpv[:, :, 1])
                        rr = zp_.tile([128, 2], F32, tag="rr")
                        nc.vector.reciprocal(rr[:], zs[:])
                        sc2 = zp_.tile([128, 1], F32, tag="sc2")
                        nc.vector.tensor_mul(sc2[:], zs[:, 0:1], rr[:, 1:2])
                        nc.vector.tensor_mul(sc2[:], sc2[:], lamn_sb[:])
                        pend = (h, u, t0, e1, e2, rr, sc2)
                        done = combine_stage(pend)
                        done_at = at_sb
                    # A @ V for this t-group; the last t-sub's transpose
                    # groups are interleaved just ahead of the AV chunks
                    # that consume them
                    u_ps = ups.tile([128, 512], F32, tag="ups")
                    for ch in range(16):
                        if ch % 4 == 0:
                            transpose_group(done, done_at, ch // 4)
                        nc.tensor.matmul(
                            u_ps[:],
                            lhsT=_r(v_sb[:, ch * 256 + h * 128:ch * 256 + (h + 1) * 128]),
                            rhs=_r(at_sb[:, ch * 512:(ch + 1) * 512]),
                            start=(ch == 0),
                            stop=(ch == 15),
                        )
                    done = None
                    nc.vector.tensor_copy(attnT[h][:, g * 512:(g + 1) * 512], u_ps[:])
                    rms_reduce_slice(h, g)


            rms_apply()

            # out_p = normed @ Wo_slice
            for tt in range(16):
                for n in range(4):
                    o_ps = ups.tile([128, 512], F32, tag="ups")
                    for m in range(2):
                        nc.tensor.matmul(
                            o_ps[:],
                            lhsT=_r(attnT[m][:, tt * 128:(tt + 1) * 128]),
                            rhs=_r(wo_sb[m][:, n * 512:(n + 1) * 512]),
                            start=(m == 0),
                            stop=(m == 1),
                        )
                    o_sb = osb.tile([128, 512], F32, tag="o_sb")
                    psum_copy(o_sb[:], o_ps[:])
                    nc.gpsimd.dma_start(
                        out_p[tt * 128:(tt + 1) * 128, n * 512:(n + 1) * 512],
                        o_sb[:],
                    )
